# revision 13
# baseline (speedup 1.0000x reference)
"""Multi-head self-attention (b=2, n=2048, emb=1024, heads=16) on 8 trn2 cores.

Sharding: core c = (b, hg) with b = c // 4, hg = c % 4. Data parallel over
batch, tensor parallel over head-groups (4 heads / 256 emb-cols per core).
Each core computes Q/K/V projections for its heads, full attention for its
heads, and a partial output projection ctx_hg @ Wo[:, hg_slice].T of shape
[2048, 1024]. The host sums the 4 partials per batch (Megatron row-parallel
reduce done on host) and adds the rank-1 bias term bv @ Wo.T + bo.

Device layout notes:
- Host pre-transposes x -> xT [emb, n] and weight slices so every matmul
  contracts over the partition dim.
- Q^T, K^T are produced in [dq, n] layout (dq = head-major), V in natural
  [n, dv] layout augmented with a ones column per head -> the ctx matmul
  ctxT[65, nq] = V_aug^T @ E^T produces softmax row-sums in row 64 for free.
- exp(S^T) runs on ACT straight out of PSUM in up-to-1536-wide instructions;
  softmax normalization is deferred to the small ctx^T tile.
- All matmuls run as float32r (full PE rate at free-dim >= 256).
- q/k biases are added on-device (free, fused into the PSUM->SBUF copy);
  v/o biases are exactly the rank-1 host-side term above.
"""

import os
import sys

for _p in ("/opt/trn_rl_repo", "/root/.axon_site/_ro/trn_rl_repo"):
    if os.path.isdir(_p) and _p not in sys.path:
        sys.path.append(_p)

import numpy as np

import concourse.bass as bass  # noqa: F401  (engine types pulled via nc)
import concourse.mybir as mybir
import concourse.tile as tile
from concourse import bacc
from concourse.bass_utils import run_bass_kernel_spmd

B, N, EMB, HEADS, HD = 2, 2048, 1024, 16, 64
N_CORES = 8
TP = 4                      # head-group shards per batch
DQ = EMB // TP              # 256 emb-cols (4 heads) per core
SCALE = HD ** -0.5          # 0.125

F32 = mybir.dt.float32
F32R = mybir.dt.float32r
FP = mybir.ActivationFunctionType

NQ = 512                    # nq chunk for projections / out-proj (moving free dim)
NJ = N // NQ                # 4 nq chunks
NQA = 256                   # nq chunk for attention (so 6 nk-chunks fit one exp)
NJA = N // NQA              # 8 attention nq chunks
NKC = 128                   # nk chunk (ctx contraction)
NT = N // NKC               # 16 nk chunks
KC = EMB // 128             # 8 e chunks
# nk-chunk groups per exp instruction. PSUM budget (8 banks): pp 1 + s0 3 +
# s1 2 + c0 1 + c1 1, so head 0 gets 1536-wide exps and head 1 1024-wide.
T_GROUPS_H = (
    [tuple(range(0, 6)), tuple(range(6, 12)), tuple(range(12, 16))],
    [tuple(range(0, 4)), tuple(range(4, 8)), tuple(range(8, 12)),
     tuple(range(12, 16))],
)


def build_program():
    """Build + compile the single SPMD program all 8 cores run."""
    nc = bacc.Bacc("TRN2", target_bir_lowering=False, debug=False,
                   num_devices=N_CORES)

    xT = nc.dram_tensor("xT", [EMB, N], F32R, kind="ExternalInput").ap()
    wqT = nc.dram_tensor("wqT", [EMB, DQ], F32R, kind="ExternalInput").ap()
    wkT = nc.dram_tensor("wkT", [EMB, DQ], F32R, kind="ExternalInput").ap()
    wvT = nc.dram_tensor("wvT", [EMB, DQ], F32R, kind="ExternalInput").ap()
    woT = nc.dram_tensor("woT", [DQ, EMB], F32R, kind="ExternalInput").ap()
    bqd = nc.dram_tensor("bq_s", [DQ], F32, kind="ExternalInput").ap()
    bkd = nc.dram_tensor("bk_s", [DQ], F32, kind="ExternalInput").ap()
    out_part = nc.dram_tensor("out_part", [N, EMB], F32,
                              kind="ExternalOutput").ap()

    with tile.TileContext(nc) as tc:
        with (
            tc.tile_pool(name="const", bufs=1) as const,
            tc.tile_pool(name="xp", bufs=12) as xp,
            tc.tile_pool(name="persist", bufs=1) as persist,
            tc.tile_pool(name="epool", bufs=2) as epool,
            tc.tile_pool(name="npool", bufs=2) as npool,
            tc.tile_pool(name="opool", bufs=3) as opool,
            # PSUM static budget (8 banks): pp 1 + s0 3 + s1 3 + c 1
            tc.tile_pool(name="ppool", bufs=1, space="PSUM") as ppool,
            tc.tile_pool(name="spool", bufs=1, space="PSUM") as spool,
            tc.tile_pool(name="cpool", bufs=1, space="PSUM") as cpool,
        ):
            # ---- constants ----
            wq_sb = const.tile([128, KC, DQ], F32R, tag="wq")
            nc.sync.dma_start(out=wq_sb, in_=wqT.rearrange("(k p) d -> p k d", p=128))
            wk_sb = const.tile([128, KC, DQ], F32R, tag="wk")
            nc.sync.dma_start(out=wk_sb, in_=wkT.rearrange("(k p) d -> p k d", p=128))
            wv_sb = const.tile([128, KC, DQ], F32R, tag="wv")
            nc.sync.dma_start(out=wv_sb, in_=wvT.rearrange("(k p) d -> p k d", p=128))
            wo_sb = const.tile([128, 2, EMB], F32R, tag="wo")
            nc.sync.dma_start(out=wo_sb, in_=woT.rearrange("(k p) e -> p k e", p=128))
            bq_sb = const.tile([128, 2], F32, tag="bq")
            nc.sync.dma_start(out=bq_sb, in_=bqd.rearrange("(m p) -> p m", p=128))
            bk_sb = const.tile([128, 2], F32, tag="bk")
            nc.sync.dma_start(out=bk_sb, in_=bkd.rearrange("(m p) -> p m", p=128))

            # ---- persistent activations ----
            qT = [persist.tile([128, N], F32R, tag=f"qT{p}", name=f"qT{p}") for p in range(2)]
            kT = [persist.tile([128, N], F32R, tag=f"kT{p}", name=f"kT{p}") for p in range(2)]
            ctxT = [persist.tile([128, N], F32R, tag=f"ctxT{p}", name=f"ctxT{p}") for p in range(2)]
            # V for all 4 local heads: [nk-part, t, head*65 + (0:64 | ones)]
            v_all = persist.tile([128, NT, 4 * (HD + 1)], F32R, tag="v")
            for h in range(4):
                nc.vector.memset(v_all[:, :, h * 65 + 64].bitcast(F32), 1.0)

            add, mult = mybir.AluOpType.add, mybir.AluOpType.mult

            # ---- projections (streamed over nq chunks of x^T) ----
            for n in range(NJ):
                xts = []
                for k in range(KC):
                    xt = xp.tile([128, NQ], F32R, tag="xt")
                    nc.sync.dma_start(
                        out=xt,
                        in_=xT[k * 128:(k + 1) * 128, n * NQ:(n + 1) * NQ])
                    xts.append(xt)
                # K then Q (attention needs full K before Q chunk j is useful)
                for wsb, bsb, dst in ((wk_sb, bk_sb, kT), (wq_sb, bq_sb, qT)):
                    for m in range(2):
                        ps = ppool.tile([128, NQ], F32, tag="pp")
                        for k in range(KC):
                            nc.tensor.matmul(
                                ps, wsb[:, k, m * 128:(m + 1) * 128],
                                xts[k], start=(k == 0), stop=(k == KC - 1))
                        nc.vector.tensor_tensor(
                            out=dst[m][:, n * NQ:(n + 1) * NQ], in0=ps,
                            in1=bsb[:, m:m + 1].broadcast_to([128, NQ]), op=add)
                # V for the 4 nk chunks covered by this nq chunk
                for tl in range(4):
                    t = n * 4 + tl
                    ps = ppool.tile([128, NQ], F32, tag="pp")
                    for k in range(KC):
                        nc.tensor.matmul(
                            ps[:, 0:DQ], xts[k][:, tl * 128:(tl + 1) * 128],
                            wv_sb[:, k, :], start=(k == 0), stop=(k == KC - 1))
                    nc.vector.tensor_copy(
                        out=v_all[:, t, :].rearrange("p (h c) -> p h c", c=65)[:, :, 0:64],
                        in_=ps[:, 0:DQ].rearrange("p (h c) -> p h c", c=64))

            # ---- attention (per head-pair p, nq chunk j of 256) ----
            # Software-pipelined: ctx matmuls for group g are emitted after
            # the S/exp of group g+1, so PE always has ready work while ACT
            # (the bottleneck) streams 1536-wide exps; heads alternate as the
            # natural PSUM ping-pong for the S tiles (tags s0/s1, 3 banks ea).
            for p in range(2):
                for j in range(NJA):
                    cps = [cpool.tile([HD + 1, NQA], F32, tag=f"c{h}",
                                      name=f"c{h}") for h in range(2)]

                    def s_mms(g, h):
                        lo = 64 * h
                        sp = spool.tile([128, len(g), NQA], F32,
                                        tag=f"s{h}", name=f"s{h}")
                        for i, t in enumerate(g):
                            nc.tensor.matmul(
                                sp[:, i, :],
                                kT[p][lo:lo + 64, t * 128:(t + 1) * 128],
                                qT[p][lo:lo + 64, j * NQA:(j + 1) * NQA],
                                start=True, stop=True)
                        return sp

                    def exp_act(sp, g, h):
                        e = epool.tile([128, len(g), NQA], F32R,
                                       tag=f"e{h}", name=f"e{h}")
                        nc.scalar.activation(e, sp, FP.Exp, scale=SCALE)
                        return e

                    def ctx_mms(e, g, h):
                        hloc = 2 * p + h
                        for i, t in enumerate(g):
                            nc.tensor.matmul(
                                cps[h],
                                v_all[:, t, hloc * 65:(hloc + 1) * 65],
                                e[:, i, :],
                                start=(t == 0), stop=(t == NT - 1))

                    # interleave the two heads' group streams; ctx trails by
                    # one work item so PE always has ready matmuls queued
                    work = []
                    for gi in range(max(len(T_GROUPS_H[0]), len(T_GROUPS_H[1]))):
                        for h in range(2):
                            if gi < len(T_GROUPS_H[h]):
                                work.append((T_GROUPS_H[h][gi], h))
                    prev = None
                    for g, h in work:
                        sp = s_mms(g, h)
                        cur = (exp_act(sp, g, h), g, h)
                        if prev is not None:
                            ctx_mms(*prev)
                        prev = cur
                    ctx_mms(*prev)

                    # normalize: ctx^T[0:64] * (1 / rowsum); rowsum in row 64
                    for h in range(2):
                        rs = npool.tile([1, NQA], F32, tag="rs", name="rs")
                        nc.vector.tensor_copy(rs, cps[h][64:65, :])
                        rb = npool.tile([64, NQA], F32, tag="rb", name="rb")
                        nc.gpsimd.partition_broadcast(rb, rs)
                        rc = npool.tile([64, NQA], F32, tag="rc", name="rc")
                        nc.vector.reciprocal(rc, rb)
                        nc.vector.tensor_tensor(
                            out=ctxT[p][h * 64:(h + 1) * 64,
                                        j * NQA:(j + 1) * NQA],
                            in0=cps[h][0:64, :], in1=rc, op=mult)

            # ---- partial output projection ----
            # PSUM via the (now idle) s0/s1 tags, alternated for double-buffering.
            for m in range(NT):
                o = opool.tile([128, EMB], F32, tag="o", name="o")
                for eo in range(2):
                    ps = spool.tile([128, NQ], F32, tag=f"s{eo}", name="po")
                    for kp in range(2):
                        nc.tensor.matmul(
                            ps, ctxT[kp][:, m * 128:(m + 1) * 128],
                            wo_sb[:, kp, eo * NQ:(eo + 1) * NQ],
                            start=(kp == 0), stop=(kp == 1))
                    nc.vector.tensor_copy(o[:, eo * NQ:(eo + 1) * NQ], ps)
                nc.sync.dma_start(out=out_part[m * 128:(m + 1) * 128, :], in_=o)

    nc.compile()
    return nc


_NC_CACHE = {}


def _get_program():
    if "nc" not in _NC_CACHE:
        _NC_CACHE["nc"] = build_program()
    return _NC_CACHE["nc"]


def make_in_maps(x, Wq, bq, Wk, bk, Wv, bv, Wo, bo):
    x = np.asarray(x, np.float32)
    xTs = [np.ascontiguousarray(x[b].T) for b in range(B)]
    in_maps = []
    for c in range(N_CORES):
        b, hg = divmod(c, TP)
        sl = slice(hg * DQ, (hg + 1) * DQ)
        in_maps.append({
            "xT": xTs[b],
            "wqT": np.ascontiguousarray(np.asarray(Wq, np.float32)[sl, :].T),
            "wkT": np.ascontiguousarray(np.asarray(Wk, np.float32)[sl, :].T),
            "wvT": np.ascontiguousarray(np.asarray(Wv, np.float32)[sl, :].T),
            "woT": np.ascontiguousarray(np.asarray(Wo, np.float32)[:, sl].T),
            "bq_s": np.ascontiguousarray(np.asarray(bq, np.float32)[sl]),
            "bk_s": np.ascontiguousarray(np.asarray(bk, np.float32)[sl]),
        })
    return in_maps


def assemble_output(results, Wv_bias_term):
    out = np.empty((B, N, EMB), np.float32)
    for b in range(B):
        acc = results[b * TP]["out_part"].astype(np.float32)
        for g in range(1, TP):
            acc = acc + results[b * TP + g]["out_part"]
        out[b] = acc + Wv_bias_term
    return out


def kernel(x, Wq, bq, Wk, bk, Wv, bv, Wo, bo):
    nc = _get_program()
    in_maps = make_in_maps(x, Wq, bq, Wk, bk, Wv, bv, Wo, bo)
    res = run_bass_kernel_spmd(nc, in_maps, list(range(N_CORES)))
    bias_term = (np.asarray(bv, np.float32) @ np.asarray(Wo, np.float32).T
                 + np.asarray(bo, np.float32))
    return assemble_output(res.results, bias_term)


# revision 14
# speedup vs baseline: 1.1277x; 1.1277x over previous
"""Multi-head self-attention (b=2, n=2048, emb=1024, heads=16) on 8 trn2 cores.

Sharding: core c = (b, hg) with b = c // 4, hg = c % 4. Data parallel over
batch, tensor parallel over head-groups (4 heads / 256 emb-cols per core).
Each core computes Q/K/V projections for its heads, full attention for its
heads, and a partial output projection ctx_hg @ Wo[:, hg_slice].T of shape
[2048, 1024]. The host sums the 4 partials per batch (Megatron row-parallel
reduce done on host) and adds the rank-1 bias term bv @ Wo.T + bo.

Device layout notes:
- Host pre-transposes x -> xT [emb, n] and weight slices so every matmul
  contracts over the partition dim.
- Q^T, K^T are produced in [dq, n] layout (dq = head-major), V in natural
  [n, dv] layout augmented with a ones column per head -> the ctx matmul
  ctxT[65, nq] = V_aug^T @ E^T produces softmax row-sums in row 64 for free.
- exp(S^T) runs on ACT straight out of PSUM in up-to-1536-wide instructions;
  softmax normalization is deferred to the small ctx^T tile.
- All matmuls run in float16 (1 cyc/col on PE + fast weight load; 10-bit
  mantissa keeps the overall error ~7e-4 scale-relative, validated vs fp32).
- q/k biases are added on-device (free, fused into the PSUM->SBUF copy);
  v/o biases are exactly the rank-1 host-side term above.
"""

import os
import sys

for _p in ("/opt/trn_rl_repo", "/root/.axon_site/_ro/trn_rl_repo"):
    if os.path.isdir(_p) and _p not in sys.path:
        sys.path.append(_p)

import numpy as np

import concourse.bass as bass  # noqa: F401  (engine types pulled via nc)
import concourse.mybir as mybir
import concourse.tile as tile
from concourse import bacc
from concourse.bass_utils import run_bass_kernel_spmd

B, N, EMB, HEADS, HD = 2, 2048, 1024, 16, 64
N_CORES = 8
TP = 4                      # head-group shards per batch
DQ = EMB // TP              # 256 emb-cols (4 heads) per core
SCALE = HD ** -0.5          # 0.125

F32 = mybir.dt.float32
F16 = mybir.dt.float16
FP = mybir.ActivationFunctionType

NQ = 512                    # nq chunk for projections / out-proj (moving free dim)
NJ = N // NQ                # 4 nq chunks
NQA = 256                   # nq chunk for attention (so 6 nk-chunks fit one exp)
NJA = N // NQA              # 8 attention nq chunks
NKC = 128                   # nk chunk (ctx contraction)
NT = N // NKC               # 16 nk chunks
KC = EMB // 128             # 8 e chunks
# nk-chunk groups per exp instruction. PSUM budget (8 banks): pp 1 + s0 3 +
# s1 2 + c0 1 + c1 1, so head 0 gets 1536-wide exps and head 1 1024-wide.
T_GROUPS_H = (
    [tuple(range(0, 6)), tuple(range(6, 12)), tuple(range(12, 16))],
    [tuple(range(0, 4)), tuple(range(4, 8)), tuple(range(8, 12)),
     tuple(range(12, 16))],
)


def build_program():
    """Build + compile the single SPMD program all 8 cores run."""
    nc = bacc.Bacc("TRN2", target_bir_lowering=False, debug=False,
                   num_devices=N_CORES)

    xT = nc.dram_tensor("xT", [EMB, N], F16, kind="ExternalInput").ap()
    wqT = nc.dram_tensor("wqT", [EMB, DQ], F16, kind="ExternalInput").ap()
    wkT = nc.dram_tensor("wkT", [EMB, DQ], F16, kind="ExternalInput").ap()
    wvT = nc.dram_tensor("wvT", [EMB, DQ], F16, kind="ExternalInput").ap()
    woT = nc.dram_tensor("woT", [DQ, EMB], F16, kind="ExternalInput").ap()
    bqd = nc.dram_tensor("bq_s", [DQ], F32, kind="ExternalInput").ap()
    bkd = nc.dram_tensor("bk_s", [DQ], F32, kind="ExternalInput").ap()
    out_part = nc.dram_tensor("out_part", [N, EMB], F32,
                              kind="ExternalOutput").ap()

    with tile.TileContext(nc) as tc:
        with (
            tc.tile_pool(name="const", bufs=1) as const,
            tc.tile_pool(name="xp", bufs=12) as xp,
            tc.tile_pool(name="persist", bufs=1) as persist,
            tc.tile_pool(name="epool", bufs=2) as epool,
            tc.tile_pool(name="npool", bufs=2) as npool,
            tc.tile_pool(name="opool", bufs=3) as opool,
            # PSUM static budget (8 banks): pp 1 + s0 3 + s1 3 + c 1
            tc.tile_pool(name="ppool", bufs=1, space="PSUM") as ppool,
            tc.tile_pool(name="spool", bufs=1, space="PSUM") as spool,
            tc.tile_pool(name="cpool", bufs=1, space="PSUM") as cpool,
        ):
            # ---- constants ----
            wq_sb = const.tile([128, KC, DQ], F16, tag="wq")
            nc.sync.dma_start(out=wq_sb, in_=wqT.rearrange("(k p) d -> p k d", p=128))
            wk_sb = const.tile([128, KC, DQ], F16, tag="wk")
            nc.sync.dma_start(out=wk_sb, in_=wkT.rearrange("(k p) d -> p k d", p=128))
            wv_sb = const.tile([128, KC, DQ], F16, tag="wv")
            nc.sync.dma_start(out=wv_sb, in_=wvT.rearrange("(k p) d -> p k d", p=128))
            wo_sb = const.tile([128, 2, EMB], F16, tag="wo")
            nc.sync.dma_start(out=wo_sb, in_=woT.rearrange("(k p) e -> p k e", p=128))
            bq_sb = const.tile([128, 2], F32, tag="bq")
            nc.sync.dma_start(out=bq_sb, in_=bqd.rearrange("(m p) -> p m", p=128))
            bk_sb = const.tile([128, 2], F32, tag="bk")
            nc.sync.dma_start(out=bk_sb, in_=bkd.rearrange("(m p) -> p m", p=128))

            # ---- persistent activations ----
            qT = [persist.tile([128, N], F16, tag=f"qT{p}", name=f"qT{p}") for p in range(2)]
            kT = [persist.tile([128, N], F16, tag=f"kT{p}", name=f"kT{p}") for p in range(2)]
            ctxT = [persist.tile([128, N], F16, tag=f"ctxT{p}", name=f"ctxT{p}") for p in range(2)]
            # V for all 4 local heads: [nk-part, t, head*65 + (0:64 | ones)]
            v_all = persist.tile([128, NT, 4 * (HD + 1)], F16, tag="v")
            for h in range(4):
                nc.vector.memset(v_all[:, :, h * 65 + 64], 1.0)

            add, mult = mybir.AluOpType.add, mybir.AluOpType.mult

            # ---- projections (streamed over nq chunks of x^T) ----
            for n in range(NJ):
                xts = []
                for k in range(KC):
                    xt = xp.tile([128, NQ], F16, tag="xt")
                    nc.sync.dma_start(
                        out=xt,
                        in_=xT[k * 128:(k + 1) * 128, n * NQ:(n + 1) * NQ])
                    xts.append(xt)
                # K then Q (attention needs full K before Q chunk j is useful)
                for wsb, bsb, dst in ((wk_sb, bk_sb, kT), (wq_sb, bq_sb, qT)):
                    for m in range(2):
                        ps = ppool.tile([128, NQ], F32, tag="pp")
                        for k in range(KC):
                            nc.tensor.matmul(
                                ps, wsb[:, k, m * 128:(m + 1) * 128],
                                xts[k], start=(k == 0), stop=(k == KC - 1))
                        nc.vector.tensor_tensor(
                            out=dst[m][:, n * NQ:(n + 1) * NQ], in0=ps,
                            in1=bsb[:, m:m + 1].broadcast_to([128, NQ]), op=add)
                # V for the 4 nk chunks covered by this nq chunk
                for tl in range(4):
                    t = n * 4 + tl
                    ps = ppool.tile([128, NQ], F32, tag="pp")
                    for k in range(KC):
                        nc.tensor.matmul(
                            ps[:, 0:DQ], xts[k][:, tl * 128:(tl + 1) * 128],
                            wv_sb[:, k, :], start=(k == 0), stop=(k == KC - 1))
                    nc.vector.tensor_copy(
                        out=v_all[:, t, :].rearrange("p (h c) -> p h c", c=65)[:, :, 0:64],
                        in_=ps[:, 0:DQ].rearrange("p (h c) -> p h c", c=64))

            # ---- attention (per head-pair p, nq chunk j of 256) ----
            # Software-pipelined: ctx matmuls for group g are emitted after
            # the S/exp of group g+1, so PE always has ready work while ACT
            # (the bottleneck) streams 1536-wide exps; heads alternate as the
            # natural PSUM ping-pong for the S tiles (tags s0/s1, 3 banks ea).
            for p in range(2):
                for j in range(NJA):
                    cps = [cpool.tile([HD + 1, NQA], F32, tag=f"c{h}",
                                      name=f"c{h}") for h in range(2)]

                    def s_mms(g, h):
                        lo = 64 * h
                        sp = spool.tile([128, len(g), NQA], F32,
                                        tag=f"s{h}", name=f"s{h}")
                        for i, t in enumerate(g):
                            nc.tensor.matmul(
                                sp[:, i, :],
                                kT[p][lo:lo + 64, t * 128:(t + 1) * 128],
                                qT[p][lo:lo + 64, j * NQA:(j + 1) * NQA],
                                start=True, stop=True)
                        return sp

                    def exp_act(sp, g, h):
                        e = epool.tile([128, len(g), NQA], F16,
                                       tag=f"e{h}", name=f"e{h}")
                        nc.scalar.activation(e, sp, FP.Exp, scale=SCALE)
                        return e

                    def ctx_mms(e, g, h):
                        hloc = 2 * p + h
                        for i, t in enumerate(g):
                            nc.tensor.matmul(
                                cps[h],
                                v_all[:, t, hloc * 65:(hloc + 1) * 65],
                                e[:, i, :],
                                start=(t == 0), stop=(t == NT - 1))

                    # interleave the two heads' group streams; ctx trails by
                    # one work item so PE always has ready matmuls queued
                    work = []
                    for gi in range(max(len(T_GROUPS_H[0]), len(T_GROUPS_H[1]))):
                        for h in range(2):
                            if gi < len(T_GROUPS_H[h]):
                                work.append((T_GROUPS_H[h][gi], h))
                    prev = None
                    for g, h in work:
                        sp = s_mms(g, h)
                        cur = (exp_act(sp, g, h), g, h)
                        if prev is not None:
                            ctx_mms(*prev)
                        prev = cur
                    ctx_mms(*prev)

                    # normalize: ctx^T[0:64] * (1 / rowsum); rowsum in row 64
                    for h in range(2):
                        rs = npool.tile([1, NQA], F32, tag="rs", name="rs")
                        nc.vector.tensor_copy(rs, cps[h][64:65, :])
                        rb = npool.tile([64, NQA], F32, tag="rb", name="rb")
                        nc.gpsimd.partition_broadcast(rb, rs)
                        rc = npool.tile([64, NQA], F32, tag="rc", name="rc")
                        nc.vector.reciprocal(rc, rb)
                        nc.vector.tensor_tensor(
                            out=ctxT[p][h * 64:(h + 1) * 64,
                                        j * NQA:(j + 1) * NQA],
                            in0=cps[h][0:64, :], in1=rc, op=mult)

            # ---- partial output projection ----
            # PSUM via the (now idle) s0/s1 tags, alternated for double-buffering.
            for m in range(NT):
                o = opool.tile([128, EMB], F32, tag="o", name="o")
                for eo in range(2):
                    ps = spool.tile([128, NQ], F32, tag=f"s{eo}", name="po")
                    for kp in range(2):
                        nc.tensor.matmul(
                            ps, ctxT[kp][:, m * 128:(m + 1) * 128],
                            wo_sb[:, kp, eo * NQ:(eo + 1) * NQ],
                            start=(kp == 0), stop=(kp == 1))
                    nc.vector.tensor_copy(o[:, eo * NQ:(eo + 1) * NQ], ps)
                nc.sync.dma_start(out=out_part[m * 128:(m + 1) * 128, :], in_=o)

    nc.compile()
    return nc


_NC_CACHE = {}


def _get_program():
    if "nc" not in _NC_CACHE:
        _NC_CACHE["nc"] = build_program()
    return _NC_CACHE["nc"]


def make_in_maps(x, Wq, bq, Wk, bk, Wv, bv, Wo, bo):
    x = np.asarray(x)
    xTs = [np.ascontiguousarray(x[b].T.astype(np.float16)) for b in range(B)]
    in_maps = []
    for c in range(N_CORES):
        b, hg = divmod(c, TP)
        sl = slice(hg * DQ, (hg + 1) * DQ)
        in_maps.append({
            "xT": xTs[b],
            "wqT": np.ascontiguousarray(np.asarray(Wq, np.float16)[sl, :].T),
            "wkT": np.ascontiguousarray(np.asarray(Wk, np.float16)[sl, :].T),
            "wvT": np.ascontiguousarray(np.asarray(Wv, np.float16)[sl, :].T),
            "woT": np.ascontiguousarray(np.asarray(Wo, np.float16)[:, sl].T),
            "bq_s": np.ascontiguousarray(np.asarray(bq, np.float32)[sl]),
            "bk_s": np.ascontiguousarray(np.asarray(bk, np.float32)[sl]),
        })
    return in_maps


def assemble_output(results, Wv_bias_term):
    out = np.empty((B, N, EMB), np.float32)
    for b in range(B):
        acc = results[b * TP]["out_part"].astype(np.float32)
        for g in range(1, TP):
            acc = acc + results[b * TP + g]["out_part"]
        out[b] = acc + Wv_bias_term
    return out


def kernel(x, Wq, bq, Wk, bk, Wv, bv, Wo, bo):
    nc = _get_program()
    in_maps = make_in_maps(x, Wq, bq, Wk, bk, Wv, bv, Wo, bo)
    res = run_bass_kernel_spmd(nc, in_maps, list(range(N_CORES)))
    bias_term = (np.asarray(bv, np.float32) @ np.asarray(Wo, np.float32).T
                 + np.asarray(bo, np.float32))
    return assemble_output(res.results, bias_term)


# revision 19
# speedup vs baseline: 1.4579x; 1.2928x over previous
"""Multi-head self-attention (b=2, n=2048, emb=1024, heads=16) on 8 trn2 cores.

Sharding: core c = (b, hg) with b = c // 4, hg = c % 4. Data parallel over
batch, tensor parallel over head-groups (4 heads / 256 emb-cols per core).
Each core computes Q/K/V projections for its heads, full attention for its
heads, and a partial output projection ctx_hg @ Wo[:, hg_slice].T of shape
[2048, 1024]. The host sums the 4 partials per batch (Megatron row-parallel
reduce done on host) and adds the rank-1 bias term bv @ Wo.T + bo.

Device layout notes:
- Host pre-transposes x -> xT [emb, n] and weight slices so every matmul
  contracts over the partition dim.
- Q^T, K^T are produced in [dq, n] layout (dq = head-major), V in natural
  [n, dv] layout augmented with a ones column per head -> the ctx matmul
  ctxT[65, nq] = V_aug^T @ E^T produces softmax row-sums in row 64 for free.
- exp(S^T) runs on ACT straight out of PSUM in up-to-1536-wide instructions;
  softmax normalization is deferred to the small ctx^T tile.
- All matmuls run in float16 (1 cyc/col on PE + fast weight load; 10-bit
  mantissa keeps the overall error ~7e-4 scale-relative, validated vs fp32).
- q/k biases are added on-device (free, fused into the PSUM->SBUF copy);
  v/o biases are exactly the rank-1 host-side term above.
"""

import os
import sys

for _p in ("/opt/trn_rl_repo", "/root/.axon_site/_ro/trn_rl_repo"):
    if os.path.isdir(_p) and _p not in sys.path:
        sys.path.append(_p)

import numpy as np

import concourse.bass as bass  # noqa: F401  (engine types pulled via nc)
import concourse.mybir as mybir
import concourse.tile as tile
from concourse import bacc
from concourse.bass_utils import run_bass_kernel_spmd

B, N, EMB, HEADS, HD = 2, 2048, 1024, 16, 64
N_CORES = 8
TP = 4                      # head-group shards per batch
DQ = EMB // TP              # 256 emb-cols (4 heads) per core
SCALE = HD ** -0.5          # 0.125

F32 = mybir.dt.float32
F16 = mybir.dt.float16
FP = mybir.ActivationFunctionType

NQ = 512                    # nq chunk for projections / out-proj (moving free dim)
NJ = N // NQ                # 4 nq chunks
NQA = 256                   # nq chunk for attention (so 6 nk-chunks fit one exp)
NJA = N // NQA              # 8 attention nq chunks
NKC = 128                   # nk chunk (ctx contraction)
NT = N // NKC               # 16 nk chunks
KC = EMB // 128             # 8 e chunks
# nk-chunk groups per exp instruction. PSUM budget (8 banks): pp 1 + s0 3 +
# s1 2 + c0 1 + c1 1, so head 0 gets 1536-wide exps and head 1 1024-wide.
T_GROUPS_H = (
    [tuple(range(0, 6)), tuple(range(6, 12)), tuple(range(12, 16))],
    [tuple(range(0, 4)), tuple(range(4, 8)), tuple(range(8, 12)),
     tuple(range(12, 16))],
)


def build_program():
    """Build + compile the single SPMD program all 8 cores run."""
    nc = bacc.Bacc("TRN2", target_bir_lowering=False, debug=False,
                   num_devices=N_CORES)

    xT = nc.dram_tensor("xT", [EMB, N], F16, kind="ExternalInput").ap()
    wqT = nc.dram_tensor("wqT", [EMB, DQ], F16, kind="ExternalInput").ap()
    wkT = nc.dram_tensor("wkT", [EMB, DQ], F16, kind="ExternalInput").ap()
    wvT = nc.dram_tensor("wvT", [EMB, DQ], F16, kind="ExternalInput").ap()
    woT = nc.dram_tensor("woT", [DQ, EMB], F16, kind="ExternalInput").ap()
    bqd = nc.dram_tensor("bq_s", [DQ], F32, kind="ExternalInput").ap()
    bkd = nc.dram_tensor("bk_s", [DQ], F32, kind="ExternalInput").ap()
    out_part = nc.dram_tensor("out_part", [N, EMB], F32,
                              kind="ExternalOutput").ap()

    with tile.TileContext(nc) as tc:
        with (
            tc.tile_pool(name="const", bufs=1) as const,
            tc.tile_pool(name="xp", bufs=12) as xp,
            tc.tile_pool(name="persist", bufs=1) as persist,
            tc.tile_pool(name="epool", bufs=2) as epool,
            tc.tile_pool(name="npool", bufs=2) as npool,
            tc.tile_pool(name="opool", bufs=NT) as opool,
            # PSUM static budget (8 banks): pp 1 + s0 3 + s1 3 + c 1
            tc.tile_pool(name="ppool", bufs=1, space="PSUM") as ppool,
            tc.tile_pool(name="spool", bufs=1, space="PSUM") as spool,
            tc.tile_pool(name="cpool", bufs=1, space="PSUM") as cpool,
        ):
            # ---- constants ----
            wq_sb = const.tile([128, KC, DQ], F16, tag="wq")
            nc.sync.dma_start(out=wq_sb, in_=wqT.rearrange("(k p) d -> p k d", p=128))
            wk_sb = const.tile([128, KC, DQ], F16, tag="wk")
            nc.sync.dma_start(out=wk_sb, in_=wkT.rearrange("(k p) d -> p k d", p=128))
            wv_sb = const.tile([128, KC, DQ], F16, tag="wv")
            nc.sync.dma_start(out=wv_sb, in_=wvT.rearrange("(k p) d -> p k d", p=128))
            wo_sb = const.tile([128, 2, EMB], F16, tag="wo")
            nc.sync.dma_start(out=wo_sb, in_=woT.rearrange("(k p) e -> p k e", p=128))
            bq_sb = const.tile([128, 2], F32, tag="bq")
            nc.sync.dma_start(out=bq_sb, in_=bqd.rearrange("(m p) -> p m", p=128))
            bk_sb = const.tile([128, 2], F32, tag="bk")
            nc.sync.dma_start(out=bk_sb, in_=bkd.rearrange("(m p) -> p m", p=128))

            # ---- persistent activations ----
            qT = [persist.tile([128, N], F16, tag=f"qT{p}", name=f"qT{p}") for p in range(2)]
            kT = [persist.tile([128, N], F16, tag=f"kT{p}", name=f"kT{p}") for p in range(2)]
            ctxT = [persist.tile([128, N], F16, tag=f"ctxT{p}", name=f"ctxT{p}") for p in range(2)]
            # V for all 4 local heads: [nk-part, t, head*65 + (0:64 | ones)]
            v_all = persist.tile([128, NT, 4 * (HD + 1)], F16, tag="v")
            for h in range(4):
                nc.vector.memset(v_all[:, :, h * 65 + 64], 1.0)

            add, mult = mybir.AluOpType.add, mybir.AluOpType.mult

            # ---- projections (streamed over nq chunks of x^T) ----
            for n in range(NJ):
                xts = []
                for k in range(KC):
                    xt = xp.tile([128, NQ], F16, tag="xt")
                    nc.sync.dma_start(
                        out=xt,
                        in_=xT[k * 128:(k + 1) * 128, n * NQ:(n + 1) * NQ])
                    xts.append(xt)
                # K then Q (attention needs full K before Q chunk j is useful)
                for wsb, bsb, dst in ((wk_sb, bk_sb, kT), (wq_sb, bq_sb, qT)):
                    for m in range(2):
                        ps = ppool.tile([128, NQ], F32, tag="pp")
                        for k in range(KC):
                            nc.tensor.matmul(
                                ps, wsb[:, k, m * 128:(m + 1) * 128],
                                xts[k], start=(k == 0), stop=(k == KC - 1))
                        nc.vector.tensor_tensor(
                            out=dst[m][:, n * NQ:(n + 1) * NQ], in0=ps,
                            in1=bsb[:, m:m + 1].broadcast_to([128, NQ]), op=add)
                # V for the 4 nk chunks covered by this nq chunk
                for tl in range(4):
                    t = n * 4 + tl
                    ps = ppool.tile([128, NQ], F32, tag="pp")
                    for k in range(KC):
                        nc.tensor.matmul(
                            ps[:, 0:DQ], xts[k][:, tl * 128:(tl + 1) * 128],
                            wv_sb[:, k, :], start=(k == 0), stop=(k == KC - 1))
                    nc.vector.tensor_copy(
                        out=v_all[:, t, :].rearrange("p (h c) -> p h c", c=65)[:, :, 0:64],
                        in_=ps[:, 0:DQ].rearrange("p (h c) -> p h c", c=64))

            # ---- attention (per head-pair p, nq chunk j of 256) ----
            # Software-pipelined: ctx matmuls for group g are emitted after
            # the S/exp of group g+1, so PE always has ready work while ACT
            # streams wide exps; heads alternate as the natural PSUM
            # ping-pong for the S tiles. The ctx PSUM bank is released by one
            # quick copy to SBUF; the reciprocal-normalize then runs fully
            # off the critical path on DVE/GpSimd.
            o_tiles = []

            def out_proj_chunk(kp, m, first):
                # one m-chunk of output-projection pass kp. Pass 0 (ctxT0)
                # is emitted interleaved with pair-1 attention as PE filler
                # (pp tag, free after projections); pass 1 runs at the end
                # (s tags, double-buffered) and adds + stores.
                if first:
                    o = opool.tile([128, EMB], F32, tag="o", name="o")
                    o_tiles.append(o)
                else:
                    o = o_tiles[m]
                for eo in range(2):
                    tag = "pp" if first else f"s{eo}"
                    po = (ppool if first else spool).tile(
                        [128, NQ], F32, tag=tag, name="po")
                    nc.tensor.matmul(
                        po, ctxT[kp][:, m * 128:(m + 1) * 128],
                        wo_sb[:, kp, eo * NQ:(eo + 1) * NQ],
                        start=True, stop=True)
                    if first:
                        nc.vector.tensor_copy(o[:, eo * NQ:(eo + 1) * NQ], po)
                    else:
                        nc.vector.tensor_tensor(
                            out=o[:, eo * NQ:(eo + 1) * NQ],
                            in0=o[:, eo * NQ:(eo + 1) * NQ], in1=po, op=add)
                if not first:
                    nc.sync.dma_start(
                        out=out_part[m * 128:(m + 1) * 128, :], in_=o)

            for p in range(2):
                for j in range(NJA):
                    cps = [cpool.tile([HD + 1, NQA], F32, tag=f"c{h}",
                                      name=f"c{h}") for h in range(2)]

                    def s_mms(g, h):
                        lo = 64 * h
                        sp = spool.tile([128, len(g), NQA], F32,
                                        tag=f"s{h}", name=f"s{h}")
                        for i, t in enumerate(g):
                            nc.tensor.matmul(
                                sp[:, i, :],
                                kT[p][lo:lo + 64, t * 128:(t + 1) * 128],
                                qT[p][lo:lo + 64, j * NQA:(j + 1) * NQA],
                                start=True, stop=True)
                        return sp

                    def exp_act(sp, g, h):
                        e = epool.tile([128, len(g), NQA], F16,
                                       tag=f"e{h}", name=f"e{h}")
                        nc.scalar.activation(e, sp, FP.Exp, scale=SCALE)
                        return e

                    def ctx_mms(e, g, h):
                        hloc = 2 * p + h
                        for i, t in enumerate(g):
                            nc.tensor.matmul(
                                cps[h],
                                v_all[:, t, hloc * 65:(hloc + 1) * 65],
                                e[:, i, :],
                                start=(t == 0), stop=(t == NT - 1))

                    # interleave the two heads' group streams; ctx trails by
                    # one work item so PE always has ready matmuls queued
                    work = []
                    for gi in range(max(len(T_GROUPS_H[0]), len(T_GROUPS_H[1]))):
                        for h in range(2):
                            if gi < len(T_GROUPS_H[h]):
                                work.append((T_GROUPS_H[h][gi], h))
                    prev = None
                    for g, h in work:
                        sp = s_mms(g, h)
                        cur = (exp_act(sp, g, h), g, h)
                        if prev is not None:
                            ctx_mms(*prev)
                        prev = cur
                    ctx_mms(*prev)

                    # normalize: ctx^T[0:64] * (1 / rowsum); rowsum in row 64.
                    # First copy out of PSUM (frees the ctx bank), then the
                    # slow reciprocal chain runs out of SBUF asynchronously.
                    for h in range(2):
                        cs = npool.tile([HD + 1, NQA], F32, tag="cs", name="cs")
                        nc.vector.tensor_copy(cs, cps[h])
                        # partition_broadcast reads physical partition 0, so
                        # stage the rowsum row there first
                        rs = npool.tile([1, NQA], F32, tag="rs", name="rs")
                        nc.vector.tensor_copy(rs, cs[64:65, :])
                        rb = npool.tile([64, NQA], F32, tag="rb", name="rb")
                        nc.gpsimd.partition_broadcast(rb, rs)
                        rc = npool.tile([64, NQA], F32, tag="rc", name="rc")
                        nc.vector.reciprocal(rc, rb)
                        nc.vector.tensor_tensor(
                            out=ctxT[p][h * 64:(h + 1) * 64,
                                        j * NQA:(j + 1) * NQA],
                            in0=cs[0:64, :], in1=rc, op=mult)
                    if p == 1:
                        # two m-chunks of out-proj pass 0 per j iteration,
                        # interleaved so PE stays dense without starving ACT
                        for m in (2 * j, 2 * j + 1):
                            out_proj_chunk(0, m, first=True)
            for m in range(NT):
                out_proj_chunk(1, m, first=False)

    nc.compile()
    return nc


_NC_CACHE = {}


def _get_program():
    if "nc" not in _NC_CACHE:
        _NC_CACHE["nc"] = build_program()
    return _NC_CACHE["nc"]


def make_in_maps(x, Wq, bq, Wk, bk, Wv, bv, Wo, bo):
    x = np.asarray(x)
    xTs = [np.ascontiguousarray(x[b].T.astype(np.float16)) for b in range(B)]
    in_maps = []
    for c in range(N_CORES):
        b, hg = divmod(c, TP)
        sl = slice(hg * DQ, (hg + 1) * DQ)
        in_maps.append({
            "xT": xTs[b],
            "wqT": np.ascontiguousarray(np.asarray(Wq, np.float16)[sl, :].T),
            "wkT": np.ascontiguousarray(np.asarray(Wk, np.float16)[sl, :].T),
            "wvT": np.ascontiguousarray(np.asarray(Wv, np.float16)[sl, :].T),
            "woT": np.ascontiguousarray(np.asarray(Wo, np.float16)[:, sl].T),
            "bq_s": np.ascontiguousarray(np.asarray(bq, np.float32)[sl]),
            "bk_s": np.ascontiguousarray(np.asarray(bk, np.float32)[sl]),
        })
    return in_maps


def assemble_output(results, Wv_bias_term):
    out = np.empty((B, N, EMB), np.float32)
    for b in range(B):
        acc = results[b * TP]["out_part"].astype(np.float32)
        for g in range(1, TP):
            acc = acc + results[b * TP + g]["out_part"]
        out[b] = acc + Wv_bias_term
    return out


def kernel(x, Wq, bq, Wk, bk, Wv, bv, Wo, bo):
    nc = _get_program()
    in_maps = make_in_maps(x, Wq, bq, Wk, bk, Wv, bv, Wo, bo)
    res = run_bass_kernel_spmd(nc, in_maps, list(range(N_CORES)))
    bias_term = (np.asarray(bv, np.float32) @ np.asarray(Wo, np.float32).T
                 + np.asarray(bo, np.float32))
    return assemble_output(res.results, bias_term)
